# revision 1
# baseline (speedup 1.0000x reference)
"""Trainium2 Bass kernel for nn_LinearDiffusion (truncated Taylor expm(a) @ x).

Math: a = row-normalized symmetric scatter of per-head edge weights onto an
(H, N, N) zero tensor; result = sum_{i=0..6} a^i x / i! with x = h reshaped
per-head.

Strategy (8 NeuronCores, one chip):
  * The adjacency is ~0.4% dense; the dense einsum would stream 1 GB of
    matrix 6x. Instead: sparse formulation with the pattern preprocessed on
    host into per-core tables.
  * Node features of all 4 heads are kept together: one node row = 64 fp32
    = 256 B, the exact granularity of `dma_gather`.
  * Shard by destination row: core k owns rows [k*1024, (k+1)*1024).
    Edge entries (r, c, w) sorted by r, padded into 128-edge chunks that
    each scatter into one 128-row block.
  * Per iteration, per core:
      1. dma_gather of x[src] rows (256 B each) from a DRAM copy of x
      2. VectorE: weighted product, split hi/lo fp16 (exact to ~2^-22)
      3. TensorE: per chunk, one-hot scatter matrix (fp8, SBUF-resident)
         x [hi|lo] rhs -> accumulate the block's (128, 128) PSUM tile
      4. evacuate PSUM, accumulate Taylor term, AllGather new x
  * Only the table *data* differs per core, so one SPMD program serves all
    8 cores; per-core tables arrive as inputs.
"""

import math
from dataclasses import dataclass

import numpy as np

import concourse.bass as bass  # noqa: F401  (kept for callers)
import concourse.tile as tile
from concourse import bacc, mybir
from concourse.bass_utils import run_bass_kernel_spmd

# ----------------------------------------------------------------- config

N, H, E, D = 8192, 4, 131072, 64
d = D // H
NCORES = 8
BLK = 128  # dst-block size == PE stationary width
K_TAYLOR = 6


@dataclass(frozen=True)
class Cfg:
    n: int = N
    n_cores: int = NCORES
    hi_lo_split: bool = True  # False -> single fp16 product (faster, ~5e-4 err)

    @property
    def rows_per_core(self):
        return self.n // self.n_cores

    @property
    def blocks_per_core(self):
        return self.rows_per_core // BLK


# ----------------------------------------------------------- preprocessing


def _entries(e, src, dst, n):
    """Unique symmetric entries with 'last write wins' duplicate semantics,
    matching jax's .at[].set() on CPU. Returns (rows, cols, w[H, nnz])."""
    src = src.astype(np.int64)
    dst = dst.astype(np.int64)
    n_edges = len(src)
    keys = np.concatenate([src * n + dst, dst * n + src])
    eid = np.concatenate([np.arange(n_edges), np.arange(n_edges)])
    order = np.arange(2 * n_edges)
    perm = np.lexsort((-order, keys))
    k_sorted = keys[perm]
    first = np.ones(len(k_sorted), dtype=bool)
    first[1:] = k_sorted[1:] != k_sorted[:-1]
    win = perm[first]
    ukeys = k_sorted[first]
    rows = (ukeys // n).astype(np.int64)
    cols = (ukeys % n).astype(np.int64)
    weids = eid[win]
    vals = e[:, weids].astype(np.float64)  # (H, nnz)
    nheads = e.shape[0]
    rowsum = np.zeros((nheads, n), dtype=np.float64)
    for hh in range(nheads):
        rowsum[hh] = np.bincount(rows, weights=vals[hh], minlength=n)
    w = (vals / rowsum[:, rows]).astype(np.float32)
    return rows, cols, w


def _make_tables(e, src, dst, cfg: Cfg):
    """Per-core device tables. Returns (tables, nch) where tables is a list
    over cores of dicts with keys idx (int16), w4 (fp32), sca (fp8)."""
    import ml_dtypes

    n = cfg.n
    rows, cols, w = _entries(e, src, dst, n)
    nheads = w.shape[0]
    bpc = cfg.blocks_per_core

    order = np.argsort(rows, kind="stable")
    rows_s, cols_s, w_s = rows[order], cols[order], w[:, order]
    blk = rows_s // BLK
    nblocks = n // BLK
    starts = np.searchsorted(blk, np.arange(nblocks + 1))
    bcnt = np.diff(starts)
    bmax = int(np.ceil(bcnt.max() / 128))  # chunks per block, uniform
    nch = bpc * bmax

    tables = []
    for k in range(cfg.n_cores):
        idx = np.zeros((nch, 128), dtype=np.int16)
        w4 = np.zeros((128, nch, nheads), dtype=np.float32)
        sca = np.zeros((128, nch, 128), dtype=ml_dtypes.float8_e4m3fn)
        for j in range(bpc):
            b = k * bpc + j
            s, cnt = starts[b], bcnt[b]
            sl = slice(s, s + cnt)
            eloc = np.arange(cnt)
            c_local = j * bmax + eloc // 128
            p = eloc % 128
            idx[c_local, p] = cols_s[sl].astype(np.int16)
            w4[p, c_local, :] = w_s[:, sl].T
            m = rows_s[sl] - b * BLK
            sca[p, c_local, m] = 1.0
        # dma_gather index layout: logical index i -> [i % 16, i // 16],
        # replicated across the 8 groups of 16 partitions.
        seq = idx.reshape(-1)  # logical order: i = c*128 + p
        wrapped = seq.reshape(-1, 16).T  # (16, nch*8)
        idx_t = np.tile(wrapped, (8, 1))  # (128, nch*8)
        tables.append(
            {
                "idx": np.ascontiguousarray(idx_t),
                "w4": np.ascontiguousarray(w4.reshape(128, nch * nheads)),
                "sca": np.ascontiguousarray(sca.reshape(128, nch * 128)),
            }
        )
    return tables, nch


# ------------------------------------------------------------ bass program

_FP32 = mybir.dt.float32
_FP16 = mybir.dt.float16
_FP8 = mybir.dt.float8e4
_I16 = mybir.dt.int16


def _build_program(cfg: Cfg, nch: int):
    n = cfg.n
    bpc = cfg.blocks_per_core
    bmax = nch // bpc
    rpc = cfg.rows_per_core
    nc = bacc.Bacc(
        "TRN2",
        target_bir_lowering=False,
        debug=False,
        num_devices=cfg.n_cores,
    )

    xin = nc.dram_tensor("xin", [n, D], _FP32, kind="ExternalInput").ap()
    x0s_d = nc.dram_tensor("x0s", [rpc, D], _FP32, kind="ExternalInput").ap()
    idx_d = nc.dram_tensor("idx", [128, nch * 8], _I16, kind="ExternalInput").ap()
    w4_d = nc.dram_tensor("w4", [128, nch * H], _FP32, kind="ExternalInput").ap()
    sca_d = nc.dram_tensor("sca", [128, nch * 128], _FP8, kind="ExternalInput").ap()
    out_d = nc.dram_tensor("out", [rpc, D], _FP32, kind="ExternalOutput").ap()

    xall = nc.dram_tensor("xall", [n, D], _FP32, addr_space="Shared").ap()
    slice_in = nc.dram_tensor("slice_in", [rpc, D], _FP32).ap()

    groups = [list(range(cfg.n_cores))]

    # Sub-batch the per-iteration work so each dma_gather stays under the
    # SWDGE descriptor-ring capacity (~9k indices per call observed safe).
    halves = 1
    while nch // halves * 128 > 9216 or bpc % halves:
        halves += 1
        assert halves <= bpc, "cannot find sub-batch split"
    hbpc = bpc // halves  # blocks per sub-batch
    hch = nch // halves  # chunks per sub-batch

    with tile.TileContext(nc) as tc:
        with (
            tc.tile_pool(name="tables", bufs=1) as tp,
            tc.tile_pool(name="xg", bufs=2) as xgp,
            tc.tile_pool(name="xgw", bufs=2) as xgwp,
            tc.tile_pool(name="acc", bufs=1) as accp,
            tc.tile_pool(name="stage", bufs=2) as stp,
            tc.tile_pool(name="psum", bufs=4, space="PSUM") as pp,
        ):
            idx_sb = tp.tile([128, nch * 8], _I16)
            w4_sb = tp.tile([128, nch, H], _FP32)
            sca_sb = tp.tile([128, nch * 128], _FP8)
            nc.sync.dma_start(out=idx_sb[:], in_=idx_d)
            nc.sync.dma_start(
                out=w4_sb[:].rearrange("p c h -> p (c h)"), in_=w4_d
            )
            nc.sync.dma_start(out=sca_sb[:], in_=sca_d)

            # x0: full copy into the gather buffer + this core's slice into
            # the running Taylor accumulator (identity term).
            nc.sync.dma_start(out=xall, in_=xin)
            result = accp.tile([128, bpc, D], _FP32)
            nc.sync.dma_start(
                out=result[:],
                in_=x0s_d.rearrange("(j p) f -> p j f", p=128),
            )

            for it in range(1, K_TAYLOR + 1):
                coef = 1.0 / math.factorial(it)
                xnext = stp.tile([128, bpc, D], _FP32, tag="xnext")
                for hf in range(halves):
                    c0 = hf * hch
                    xg = xgp.tile([128, hch, D], _FP32, tag="xg")
                    nc.gpsimd.dma_gather(
                        xg[:],
                        xall,
                        idx_sb[:, c0 * 8 : (c0 + hch) * 8],
                        hch * 128,
                        hch * 128,
                        D,
                        single_packet=False,
                    )
                    # prod = xg * w4 (broadcast each head weight over d)
                    xg4 = xg[:].rearrange("p c (h f) -> p c h f", h=H)
                    w4v = (
                        w4_sb[:, c0 : c0 + hch, :]
                        .unsqueeze(3)
                        .to_broadcast([128, hch, H, d])
                    )
                    xgw = xgwp.tile([128, hch, 2 * D], _FP16, tag="xgw")
                    hi = xgw[:, :, 0:D].rearrange("p c (h f) -> p c h f", h=H)
                    lo = xgw[:, :, D : 2 * D].rearrange(
                        "p c (h f) -> p c h f", h=H
                    )
                    if cfg.hi_lo_split:
                        nc.vector.tensor_mul(xg4, xg4, w4v)
                        nc.scalar.copy(hi, xg4)
                        nc.vector.tensor_sub(lo, xg4, hi)
                    else:
                        nc.vector.tensor_mul(hi, xg4, w4v)
                        nc.vector.memset(xgw[:, :, D : 2 * D], 0.0)

                    for jj in range(hf * hbpc, (hf + 1) * hbpc):
                        ps = pp.tile([128, 2 * D], _FP32, tag="ps")
                        for b in range(bmax):
                            c = jj * bmax + b
                            nc.tensor.matmul(
                                ps[:],
                                lhsT=sca_sb[:, c * 128 : (c + 1) * 128],
                                rhs=xgw[:, c - c0, :],
                                start=(b == 0),
                                stop=(b == bmax - 1),
                            )
                        nc.scalar.copy(xnext[:, jj, :], ps[:, 0:D])
                        nc.vector.tensor_add(
                            xnext[:, jj, :], xnext[:, jj, :], ps[:, D : 2 * D]
                        )
                        nc.vector.scalar_tensor_tensor(
                            result[:, jj, :],
                            xnext[:, jj, :],
                            coef,
                            result[:, jj, :],
                            op0=mybir.AluOpType.mult,
                            op1=mybir.AluOpType.add,
                        )
                if it < K_TAYLOR:
                    nc.sync.dma_start(
                        out=slice_in.rearrange("(j p) f -> p j f", p=128),
                        in_=xnext[:],
                    )
                    nc.gpsimd.collective_compute(
                        "AllGather",
                        mybir.AluOpType.bypass,
                        replica_groups=groups,
                        ins=[slice_in],
                        outs=[xall],
                    )

            nc.sync.dma_start(
                out=out_d.rearrange("(j p) f -> p j f", p=128),
                in_=result[:],
            )

    nc.compile()
    return nc


# ------------------------------------------------------------------ driver

_CACHE = {}


def _get_program(cfg: Cfg, nch: int):
    key = (cfg, nch)
    if key not in _CACHE:
        _CACHE[key] = _build_program(cfg, nch)
    return _CACHE[key]


def _in_maps(x0, tables, cfg: Cfg):
    rpc = cfg.rows_per_core
    return [
        {
            "xin": x0,
            "x0s": np.ascontiguousarray(x0[k * rpc : (k + 1) * rpc]),
            "idx": t["idx"],
            "w4": t["w4"],
            "sca": t["sca"],
        }
        for k, t in enumerate(tables)
    ]


def run(h, e, src, dst, cfg: Cfg = Cfg(), trace: bool = False):
    """Full pipeline: preprocess, build/compile (cached), execute, assemble."""
    h = np.asarray(h, dtype=np.float32)
    e = np.asarray(e, dtype=np.float32)
    src = np.asarray(src)
    dst = np.asarray(dst)
    nheads = e.shape[0]
    n = h.shape[0]
    dd = h.shape[1] // nheads
    assert (n, nheads, dd) == (cfg.n, H, d), (n, nheads, dd)

    tables, nch = _make_tables(e, src, dst, cfg)
    x0 = np.ascontiguousarray(
        h.reshape(nheads, n, dd).transpose(1, 0, 2).reshape(n, nheads * dd)
    )
    nc = _get_program(cfg, nch)
    res = run_bass_kernel_spmd(
        nc,
        _in_maps(x0, tables, cfg),
        list(range(cfg.n_cores)),
        trace=trace,
    )
    out = np.concatenate(
        [res.results[k]["out"] for k in range(cfg.n_cores)], axis=0
    )
    # back to reference layout: (n, H, d) node-major -> (H, n, d) -> (N, D)
    out = np.ascontiguousarray(out.reshape(n, nheads, dd).transpose(1, 0, 2)).reshape(
        n, nheads * dd
    )
    return out, res


def kernel(h, e, src, dst):
    out, _ = run(h, e, src, dst)
    return out



# revision 3
# speedup vs baseline: 5.3930x; 5.3930x over previous
"""Trainium2 Bass kernel for nn_LinearDiffusion (truncated Taylor expm(a) @ x).

Math: a = row-normalized symmetric scatter of per-head edge weights onto an
(H, N, N) zero tensor; result = sum_{i=0..6} a^i x / i! with x = h reshaped
per-head.

Strategy (8 NeuronCores, one chip), v2 — all-TensorE sparse SpMM:
  * The previous version gathered x[src] rows via SWDGE dma_gather: ~8 ns
    of Q7 descriptor generation per row x 32k rows x 6 iters dominated
    (GpSimd 91% busy).  v2 performs both the gather AND the scatter as
    one-hot matmuls on the tensor engine, which this problem leaves idle.
  * Layout: x node-major in SBUF as (128, 64, 64) fp16 — partition p,
    src block J, feature f (all 4 heads concatenated, d=16 each).
  * Edges are grouped by (I = dst block within core, J = src block) into
    8*64 = 512 "cells" per core, each holding <= 128 edges (max 96 for
    this graph).  The cell grid is the same for every core, so one SPMD
    program serves all 8 cores; only the one-hot tables differ per core.
  * Per cell (I, J), slot q holds one edge (r, c, w):
      1. gather:  ps[q, f]   = sum_p goh[p, q] * x[J*128+p, f]   (TensorE)
      2. weight:  xgw[q, f]  = ps[q, f] * w4[q, head(f)]         (VectorE,
         one op per group of 8 cells, PSUM -> SBUF fp16)
      3. scatter: out[m, f] += sum_q sca[q, m] * xgw[q, f]       (TensorE,
         PSUM-accumulated over the 64 J cells of block I)
  * Software-pipelined with a 2-group lookahead so the PE never waits on
    the vector engine; everything stays HAM-warm.
  * Taylor accumulation in fp32; x travels between iterations as fp16
    through an AllGather (1 MB) + strided SBUF reload.
"""

import math
from dataclasses import dataclass

import numpy as np

import concourse.bass as bass  # noqa: F401  (kept for callers)
import concourse.tile as tile
from concourse import bacc, mybir
from concourse.bass_utils import run_bass_kernel_spmd

# ----------------------------------------------------------------- config

N, H, E, D = 8192, 4, 131072, 64
d = D // H
NCORES = 8
BLK = 128
NJ = N // BLK          # 64 src blocks
NI = N // NCORES // BLK  # 8 dst blocks per core
NCELL = NI * NJ        # 512 cells per core
GRP = 8                # cells per PSUM bank / vector op
NGRP = NCELL // GRP    # 64 groups per iteration
LOOKAHEAD = 2          # groups the gather runs ahead of the scatter
K_TAYLOR = 6


@dataclass(frozen=True)
class Cfg:
    n: int = N
    n_cores: int = NCORES
    hi_lo_split: bool = True  # accepted for test.py compat; unused in v2

    @property
    def rows_per_core(self):
        return self.n // self.n_cores


# ----------------------------------------------------------- preprocessing


def _entries(e, src, dst, n):
    """Unique symmetric entries with 'last write wins' duplicate semantics,
    matching jax's .at[].set() on CPU. Returns (rows, cols, w[H, nnz])."""
    src = src.astype(np.int64)
    dst = dst.astype(np.int64)
    n_edges = len(src)
    keys = np.concatenate([src * n + dst, dst * n + src])
    eid = np.concatenate([np.arange(n_edges), np.arange(n_edges)])
    order = np.arange(2 * n_edges)
    perm = np.lexsort((-order, keys))
    k_sorted = keys[perm]
    first = np.ones(len(k_sorted), dtype=bool)
    first[1:] = k_sorted[1:] != k_sorted[:-1]
    win = perm[first]
    ukeys = k_sorted[first]
    rows = (ukeys // n).astype(np.int64)
    cols = (ukeys % n).astype(np.int64)
    weids = eid[win]
    vals = e[:, weids].astype(np.float64)  # (H, nnz)
    nheads = e.shape[0]
    rowsum = np.zeros((nheads, n), dtype=np.float64)
    for hh in range(nheads):
        rowsum[hh] = np.bincount(rows, weights=vals[hh], minlength=n)
    w = (vals / rowsum[:, rows]).astype(np.float32)
    return rows, cols, w


def _make_tables(e, src, dst, cfg: Cfg):
    """Per-core one-hot gather/scatter tables for the (I, J) cell grid."""
    import ml_dtypes

    n = cfg.n
    rows, cols, w = _entries(e, src, dst, n)
    rpc = cfg.rows_per_core

    tables = []
    for k in range(cfg.n_cores):
        m = (rows >= k * rpc) & (rows < (k + 1) * rpc)
        r = rows[m] - k * rpc
        c = cols[m]
        wv = w[:, m]  # (H, nk)
        I = r // BLK
        J = c // BLK
        cell = I * NJ + J
        order = np.argsort(cell, kind="stable")
        r, c, wv, cell = r[order], c[order], wv[:, order], cell[order]
        starts = np.searchsorted(cell, np.arange(NCELL + 1))
        counts = np.diff(starts)
        assert counts.max() <= 128, f"cell overflow: {counts.max()}"
        slot = np.arange(len(r)) - starts[cell]

        goh = np.zeros((128, NCELL * 128), dtype=ml_dtypes.float8_e4m3fn)
        goh[c % BLK, cell * 128 + slot] = 1.0
        sca = np.zeros((128, NCELL * 128), dtype=ml_dtypes.float8_e4m3fn)
        sca[slot, cell * 128 + (r % BLK)] = 1.0
        w4 = np.zeros((128, NCELL, H), dtype=np.float32)
        w4[slot, cell, :] = wv.T
        tables.append(
            {
                "goh": goh,
                "sca": sca,
                "w4": np.ascontiguousarray(w4.reshape(128, NCELL * H)),
            }
        )
    return tables


# ------------------------------------------------------------ bass program

_FP32 = mybir.dt.float32
_FP16 = mybir.dt.float16
_FP8 = mybir.dt.float8e4


def _build_program(cfg: Cfg):
    rpc = cfg.rows_per_core
    nc = bacc.Bacc(
        "TRN2",
        target_bir_lowering=False,
        debug=False,
        num_devices=cfg.n_cores,
    )

    x0f_d = nc.dram_tensor("x0f", [N, D], _FP16, kind="ExternalInput").ap()
    x0s_d = nc.dram_tensor("x0s", [rpc, D], _FP32, kind="ExternalInput").ap()
    goh_d = nc.dram_tensor(
        "goh", [128, NCELL * 128], _FP8, kind="ExternalInput"
    ).ap()
    sca_d = nc.dram_tensor(
        "sca", [128, NCELL * 128], _FP8, kind="ExternalInput"
    ).ap()
    w4_d = nc.dram_tensor("w4", [128, NCELL * H], _FP32, kind="ExternalInput").ap()
    out_d = nc.dram_tensor("out", [rpc, D], _FP32, kind="ExternalOutput").ap()

    xg_dram = nc.dram_tensor("xg", [N, D], _FP16, addr_space="Shared").ap()
    slice_in = nc.dram_tensor("slice_in", [rpc, D], _FP16).ap()

    groups = [list(range(cfg.n_cores))]
    CPI = NJ * 128  # table columns per dst block I

    with tile.TileContext(nc) as tc:
        with (
            tc.tile_pool(name="tables", bufs=1) as tp,
            tc.tile_pool(name="xall", bufs=1) as xap,
            tc.tile_pool(name="xgw", bufs=4) as xgp,
            tc.tile_pool(name="acc", bufs=1) as accp,
            tc.tile_pool(name="xnext", bufs=2) as xnp,
            tc.tile_pool(name="psg", bufs=4, space="PSUM") as pgp,
            tc.tile_pool(name="pso", bufs=1, space="PSUM") as pop,
        ):
            goh_t = [
                tp.tile([128, CPI], _FP8, name=f"goh{i}") for i in range(NI)
            ]
            sca_t = [
                tp.tile([128, CPI], _FP8, name=f"sca{i}") for i in range(NI)
            ]
            w4_t = tp.tile([128, NCELL, H], _FP32)
            result = accp.tile([128, NI, D], _FP32)

            # I=0 tables first so compute can start while the rest stream in.
            nc.sync.dma_start(out=goh_t[0][:], in_=goh_d[:, 0:CPI])
            nc.sync.dma_start(out=sca_t[0][:], in_=sca_d[:, 0:CPI])
            nc.sync.dma_start(
                out=w4_t[:].rearrange("p c h -> p (c h)"), in_=w4_d
            )
            nc.sync.dma_start(
                out=result[:],
                in_=x0s_d.rearrange("(j p) f -> p j f", p=128),
            )
            xall0 = xap.tile([128, NJ, D], _FP16, tag="xall")
            nc.sync.dma_start(
                out=xall0[:], in_=x0f_d.rearrange("(j p) f -> p j f", p=128)
            )
            for i in range(1, NI):
                nc.sync.dma_start(
                    out=goh_t[i][:], in_=goh_d[:, i * CPI : (i + 1) * CPI]
                )
                nc.sync.dma_start(
                    out=sca_t[i][:], in_=sca_d[:, i * CPI : (i + 1) * CPI]
                )

            out_ps = pop.tile([128, NI, D], _FP32)
            xall = xall0

            for it in range(1, K_TAYLOR + 1):
                coef = 1.0 / math.factorial(it)
                if it > 1:
                    xall = xap.tile([128, NJ, D], _FP16, tag="xall")
                    nc.sync.dma_start(
                        out=xall[:],
                        in_=xg_dram.rearrange("(j p) f -> p j f", p=128),
                    )
                if it < K_TAYLOR:
                    xnext = xnp.tile([128, NI, D], _FP16, tag="xnext")

                # software-pipelined gather -> weight -> scatter over groups
                pend = [None] * NGRP  # xgw tiles awaiting scatter
                for t in range(NGRP + LOOKAHEAD):
                    if t < NGRP:
                        gI, gg = divmod(t, NGRP // NI)
                        ps_g = pgp.tile([128, GRP, D], _FP32, tag="psg")
                        for jj in range(GRP):
                            J = gg * GRP + jj
                            cc = J * 128
                            nc.tensor.matmul(
                                ps_g[:, jj, :],
                                lhsT=goh_t[gI][:, cc : cc + 128],
                                rhs=xall[:, J, :],
                                start=True,
                                stop=True,
                            )
                        xgw_g = xgp.tile([128, GRP, D], _FP16, tag="xgw")
                        c0 = gI * NJ + gg * GRP
                        wv = (
                            w4_t[:, c0 : c0 + GRP, :]
                            .unsqueeze(3)
                            .to_broadcast([128, GRP, H, d])
                        )
                        nc.vector.tensor_mul(
                            xgw_g[:].rearrange("p j (h f) -> p j h f", h=H),
                            ps_g[:].rearrange("p j (h f) -> p j h f", h=H),
                            wv,
                        )
                        pend[t] = xgw_g
                    s = t - LOOKAHEAD
                    if s < 0:
                        continue
                    sI, sg = divmod(s, NGRP // NI)
                    xgw_s = pend[s]
                    pend[s] = None
                    for jj in range(GRP):
                        J = sg * GRP + jj
                        cc = J * 128
                        nc.tensor.matmul(
                            out_ps[:, sI, :],
                            lhsT=sca_t[sI][:, cc : cc + 128],
                            rhs=xgw_s[:, jj, :],
                            start=(J == 0),
                            stop=(J == NJ - 1),
                        )
                    if sg == NGRP // NI - 1:
                        # block sI complete: Taylor accumulate + next-x cast
                        nc.vector.scalar_tensor_tensor(
                            result[:, sI, :],
                            out_ps[:, sI, :],
                            coef,
                            result[:, sI, :],
                            op0=mybir.AluOpType.mult,
                            op1=mybir.AluOpType.add,
                        )
                        if it < K_TAYLOR:
                            nc.scalar.copy(xnext[:, sI, :], out_ps[:, sI, :])

                if it < K_TAYLOR:
                    nc.sync.dma_start(
                        out=slice_in.rearrange("(j p) f -> p j f", p=128),
                        in_=xnext[:],
                    )
                    nc.gpsimd.collective_compute(
                        "AllGather",
                        mybir.AluOpType.bypass,
                        replica_groups=groups,
                        ins=[slice_in],
                        outs=[xg_dram],
                    )

            nc.sync.dma_start(
                out=out_d.rearrange("(j p) f -> p j f", p=128),
                in_=result[:],
            )

    nc.compile()
    return nc


# ------------------------------------------------------------------ driver

_CACHE = {}


def _get_program(cfg: Cfg):
    if cfg not in _CACHE:
        _CACHE[cfg] = _build_program(cfg)
    return _CACHE[cfg]


def run(h, e, src, dst, cfg: Cfg = Cfg(), trace: bool = False):
    """Full pipeline: preprocess, build/compile (cached), execute, assemble."""
    h = np.asarray(h, dtype=np.float32)
    e = np.asarray(e, dtype=np.float32)
    src = np.asarray(src)
    dst = np.asarray(dst)
    nheads = e.shape[0]
    n = h.shape[0]
    dd = h.shape[1] // nheads
    assert (n, nheads, dd) == (cfg.n, H, d), (n, nheads, dd)

    tables = _make_tables(e, src, dst, cfg)
    x0 = np.ascontiguousarray(
        h.reshape(nheads, n, dd).transpose(1, 0, 2).reshape(n, nheads * dd)
    )
    x0f = x0.astype(np.float16)
    rpc = cfg.rows_per_core
    in_maps = [
        {
            "x0f": x0f,
            "x0s": np.ascontiguousarray(x0[k * rpc : (k + 1) * rpc]),
            "goh": t["goh"],
            "sca": t["sca"],
            "w4": t["w4"],
        }
        for k, t in enumerate(tables)
    ]
    nc = _get_program(cfg)
    res = run_bass_kernel_spmd(
        nc,
        in_maps,
        list(range(cfg.n_cores)),
        trace=trace,
    )
    out = np.concatenate(
        [res.results[k]["out"] for k in range(cfg.n_cores)], axis=0
    )
    # back to reference layout: (n, H, d) node-major -> (H, n, d) -> (N, D)
    out = np.ascontiguousarray(out.reshape(n, nheads, dd).transpose(1, 0, 2)).reshape(
        n, nheads * dd
    )
    return out, res


def kernel(h, e, src, dst):
    out, _ = run(h, e, src, dst)
    return out


# revision 12
# speedup vs baseline: 5.8122x; 1.0777x over previous
"""Trainium2 Bass kernel for nn_LinearDiffusion (truncated Taylor expm(a) @ x).

Math: a = row-normalized symmetric scatter of per-head edge weights onto an
(H, N, N) zero tensor; result = sum_{i=0..6} a^i x / i! with x = h reshaped
per-head.

Strategy (8 NeuronCores, one chip), v2 — all-TensorE sparse SpMM:
  * The previous version gathered x[src] rows via SWDGE dma_gather: ~8 ns
    of Q7 descriptor generation per row x 32k rows x 6 iters dominated
    (GpSimd 91% busy).  v2 performs both the gather AND the scatter as
    one-hot matmuls on the tensor engine, which this problem leaves idle.
  * Layout: x node-major in SBUF as (128, 64, 64) fp16 — partition p,
    src block J, feature f (all 4 heads concatenated, d=16 each).
  * Edges are grouped by (I = dst block within core, J = src block) into
    8*64 = 512 "cells" per core, each holding <= 128 edges (max 96 for
    this graph).  The cell grid is the same for every core, so one SPMD
    program serves all 8 cores; only the one-hot tables differ per core.
  * Per cell (I, J), slot q holds one edge (r, c, w):
      1. gather:  ps[q, f]   = sum_p goh[p, q] * x[J*128+p, f]   (TensorE)
      2. weight:  xgw[q, f]  = ps[q, f] * w4[q, head(f)]         (VectorE,
         one op per group of 8 cells, PSUM -> SBUF fp16)
      3. scatter: out[m, f] += sum_q sca[q, m] * xgw[q, f]       (TensorE,
         PSUM-accumulated over the 64 J cells of block I)
  * Software-pipelined with a 2-group lookahead so the PE never waits on
    the vector engine; everything stays HAM-warm.
  * Taylor accumulation in fp32; x travels between iterations as fp16
    through an AllGather (1 MB) + strided SBUF reload.
"""

import math
from dataclasses import dataclass

import numpy as np

import concourse.bass as bass  # noqa: F401  (kept for callers)
import concourse.tile as tile
from concourse import bacc, mybir
from concourse.bass_utils import run_bass_kernel_spmd

# ----------------------------------------------------------------- config

N, H, E, D = 8192, 4, 131072, 64
d = D // H
NCORES = 8
BLK = 128
NJ = N // BLK          # 64 src blocks
NI = N // NCORES // BLK  # 8 dst blocks per core
NCELL = NI * NJ        # 512 cells per core
GRP = 8                # cells per PSUM bank / vector op
NGRP = NCELL // GRP    # 64 groups per iteration
LOOKAHEAD = 3          # groups the gather runs ahead of the scatter
K_TAYLOR = 6


@dataclass(frozen=True)
class Cfg:
    n: int = N
    n_cores: int = NCORES
    hi_lo_split: bool = True  # accepted for test.py compat; unused in v2

    @property
    def rows_per_core(self):
        return self.n // self.n_cores


# ----------------------------------------------------------- preprocessing


def _entries(e, src, dst, n):
    """Unique symmetric entries with 'last write wins' duplicate semantics,
    matching jax's .at[].set() on CPU. Returns (rows, cols, w[H, nnz])."""
    src = src.astype(np.int64)
    dst = dst.astype(np.int64)
    n_edges = len(src)
    keys = np.concatenate([src * n + dst, dst * n + src])
    eid = np.concatenate([np.arange(n_edges), np.arange(n_edges)])
    order = np.arange(2 * n_edges)
    perm = np.lexsort((-order, keys))
    k_sorted = keys[perm]
    first = np.ones(len(k_sorted), dtype=bool)
    first[1:] = k_sorted[1:] != k_sorted[:-1]
    win = perm[first]
    ukeys = k_sorted[first]
    rows = (ukeys // n).astype(np.int64)
    cols = (ukeys % n).astype(np.int64)
    weids = eid[win]
    vals = e[:, weids].astype(np.float64)  # (H, nnz)
    nheads = e.shape[0]
    rowsum = np.zeros((nheads, n), dtype=np.float64)
    for hh in range(nheads):
        rowsum[hh] = np.bincount(rows, weights=vals[hh], minlength=n)
    w = (vals / rowsum[:, rows]).astype(np.float32)
    return rows, cols, w


# Logical column order: the 4 blocks each core publishes in collective A
# (its dst blocks 0..3) come first, its collective-B blocks second.  This
# lets the next iteration consume the early-collective data first.
_PHYS_J = np.array(
    [8 * k + i for k in range(NCORES) for i in range(4)]
    + [8 * k + 4 + i for k in range(NCORES) for i in range(4)]
)


def _logical_j(J):
    k, i = J // NI, J % NI
    return np.where(i < 4, k * 4 + i, 32 + k * 4 + (i - 4))


def _make_tables(e, src, dst, cfg: Cfg):
    """Per-core one-hot gather/scatter tables for the (I, J) cell grid."""
    import ml_dtypes

    n = cfg.n
    rows, cols, w = _entries(e, src, dst, n)
    rpc = cfg.rows_per_core

    tables = []
    for k in range(cfg.n_cores):
        m = (rows >= k * rpc) & (rows < (k + 1) * rpc)
        r = rows[m] - k * rpc
        c = cols[m]
        wv = w[:, m]  # (H, nk)
        I = r // BLK
        J = _logical_j(c // BLK)
        cell = I * NJ + J
        order = np.argsort(cell, kind="stable")
        r, c, wv, cell = r[order], c[order], wv[:, order], cell[order]
        starts = np.searchsorted(cell, np.arange(NCELL + 1))
        counts = np.diff(starts)
        assert counts.max() <= 128, f"cell overflow: {counts.max()}"
        slot = np.arange(len(r)) - starts[cell]

        goh = np.zeros((128, NCELL * 128), dtype=ml_dtypes.float8_e4m3fn)
        goh[c % BLK, cell * 128 + slot] = 1.0
        sca = np.zeros((128, NCELL * 128), dtype=ml_dtypes.float8_e4m3fn)
        sca[slot, cell * 128 + (r % BLK)] = 1.0
        w4 = np.zeros((128, NCELL, H), dtype=np.float32)
        w4[slot, cell, :] = wv.T
        tables.append(
            {
                "goh": goh,
                "sca": sca,
                "w4": np.ascontiguousarray(w4.reshape(128, NCELL * H)),
            }
        )
    return tables


# ------------------------------------------------------------ bass program

_FP32 = mybir.dt.float32
_FP16 = mybir.dt.float16
_FP8 = mybir.dt.float8e4


def _build_program(cfg: Cfg):
    rpc = cfg.rows_per_core
    nc = bacc.Bacc(
        "TRN2",
        target_bir_lowering=False,
        debug=False,
        num_devices=cfg.n_cores,
    )

    x0p_d = nc.dram_tensor("x0p", [128, NJ * D], _FP16, kind="ExternalInput").ap()
    x0s_d = nc.dram_tensor("x0s", [rpc, D], _FP32, kind="ExternalInput").ap()
    goh_d = nc.dram_tensor(
        "goh", [128, NCELL * 128], _FP8, kind="ExternalInput"
    ).ap()
    sca_d = nc.dram_tensor(
        "sca", [128, NCELL * 128], _FP8, kind="ExternalInput"
    ).ap()
    w4_d = nc.dram_tensor("w4", [128, NCELL * H], _FP32, kind="ExternalInput").ap()
    out_d = nc.dram_tensor("out", [rpc, D], _FP32, kind="ExternalOutput").ap()

    # half-slab collective buffers, rank-major so both sides stream linearly
    HD = 4 * D  # half-slab free size per partition (4 blocks x 64 feats)
    slcA = nc.dram_tensor("slcA", [128, HD], _FP16).ap()
    slcB = nc.dram_tensor("slcB", [128, HD], _FP16).ap()
    xgA = nc.dram_tensor(
        "xgA", [NCORES, 128, HD], _FP16, addr_space="Shared"
    ).ap()
    xgB = nc.dram_tensor(
        "xgB", [NCORES, 128, HD], _FP16, addr_space="Shared"
    ).ap()
    # tiny warmup collective to absorb ncfw first-call cost (data unused)
    slcW = nc.dram_tensor("slcW", [1, 128], _FP16).ap()
    xgW = nc.dram_tensor("xgW", [NCORES, 128], _FP16, addr_space="Shared").ap()

    groups = [list(range(cfg.n_cores))]
    CPI = NJ * 128  # table columns per dst block I

    with tile.TileContext(nc) as tc:
        with (
            tc.tile_pool(name="tables", bufs=1) as tp,
            tc.tile_pool(name="xall", bufs=2) as xap,
            tc.tile_pool(name="xgw", bufs=6) as xgp,
            tc.tile_pool(name="acc", bufs=1) as accp,
            tc.tile_pool(name="xnext", bufs=2) as xnp,
            tc.tile_pool(name="psg", bufs=6, space="PSUM") as pgp,
            tc.tile_pool(name="pso", bufs=1, space="PSUM") as pop,
        ):
            goh_t = [
                tp.tile([128, CPI], _FP8, name=f"goh{i}") for i in range(NI)
            ]
            sca_t = [
                tp.tile([128, CPI], _FP8, name=f"sca{i}") for i in range(NI)
            ]
            w4_t = tp.tile([128, NCELL, H], _FP32)
            result = accp.tile([128, NI, D], _FP32)

            # I=0 tables first so compute can start while the rest stream in.
            nc.sync.dma_start(out=goh_t[0][:], in_=goh_d[:, 0:CPI])
            nc.sync.dma_start(out=sca_t[0][:], in_=sca_d[:, 0:CPI])
            nc.sync.dma_start(
                out=w4_t[:].rearrange("p c h -> p (c h)"), in_=w4_d
            )
            nc.sync.dma_start(
                out=result[:],
                in_=x0s_d.rearrange("(j p) f -> p j f", p=128),
            )
            xallA = xap.tile([128, NJ // 2, D], _FP16, tag="xa")
            xallB = xap.tile([128, NJ // 2, D], _FP16, tag="xb")
            nc.sync.dma_start(
                out=xallA[:],
                in_=x0p_d[:, 0 : NJ * D // 2].rearrange("p (j f) -> p j f", f=D),
            )
            nc.sync.dma_start(
                out=xallB[:],
                in_=x0p_d[:, NJ * D // 2 :].rearrange("p (j f) -> p j f", f=D),
            )
            for i in range(1, NI):
                nc.sync.dma_start(
                    out=goh_t[i][:], in_=goh_d[:, i * CPI : (i + 1) * CPI]
                )
                nc.sync.dma_start(
                    out=sca_t[i][:], in_=sca_d[:, i * CPI : (i + 1) * CPI]
                )

            out_ps = pop.tile([128, NI, D], _FP32)

            for it in range(1, K_TAYLOR + 1):
                coef = 1.0 / math.factorial(it)
                if it > 1:
                    xallA = xap.tile([128, NJ // 2, D], _FP16, tag="xa")
                    xallB = xap.tile([128, NJ // 2, D], _FP16, tag="xb")
                    nc.sync.dma_start(
                        out=xallA[:].rearrange("p (k i) f -> p k i f", k=NCORES),
                        in_=xgA.rearrange("k p (i f) -> p k i f", f=D),
                    )
                    nc.sync.dma_start(
                        out=xallB[:].rearrange("p (k i) f -> p k i f", k=NCORES),
                        in_=xgB.rearrange("k p (i f) -> p k i f", f=D),
                    )
                if it < K_TAYLOR:
                    xnext = xnp.tile([128, NI, D], _FP16, tag="xnext")

                # software-pipelined gather -> weight -> scatter over groups
                pend = [None] * NGRP  # xgw tiles awaiting scatter
                for t in range(NGRP + LOOKAHEAD):
                    if t < NGRP:
                        gI, gg = divmod(t, NGRP // NI)
                        ps_g = pgp.tile([128, GRP, D], _FP32, tag="psg")
                        for jj in range(GRP):
                            J = gg * GRP + jj
                            xsrc = (
                                xallA[:, J, :]
                                if J < NJ // 2
                                else xallB[:, J - NJ // 2, :]
                            )
                            cc = J * 128
                            nc.tensor.matmul(
                                ps_g[:, jj, :],
                                lhsT=goh_t[gI][:, cc : cc + 128],
                                rhs=xsrc,
                                start=True,
                                stop=True,
                            )
                        xgw_g = xgp.tile([128, GRP, D], _FP16, tag="xgw")
                        c0 = gI * NJ + gg * GRP
                        wv = (
                            w4_t[:, c0 : c0 + GRP, :]
                            .unsqueeze(3)
                            .to_broadcast([128, GRP, H, d])
                        )
                        nc.vector.tensor_mul(
                            xgw_g[:].rearrange("p j (h f) -> p j h f", h=H),
                            ps_g[:].rearrange("p j (h f) -> p j h f", h=H),
                            wv,
                        )
                        pend[t] = xgw_g
                    s = t - LOOKAHEAD
                    if s < 0:
                        continue
                    sI, sg = divmod(s, NGRP // NI)
                    xgw_s = pend[s]
                    pend[s] = None
                    for jj in range(GRP):
                        J = sg * GRP + jj
                        cc = J * 128
                        nc.tensor.matmul(
                            out_ps[:, sI, :],
                            lhsT=sca_t[sI][:, cc : cc + 128],
                            rhs=xgw_s[:, jj, :],
                            start=(J == 0),
                            stop=(J == NJ - 1),
                        )
                    if sg == NGRP // NI - 1:
                        # block sI complete: Taylor accumulate + next-x cast
                        nc.vector.scalar_tensor_tensor(
                            result[:, sI, :],
                            out_ps[:, sI, :],
                            coef,
                            result[:, sI, :],
                            op0=mybir.AluOpType.mult,
                            op1=mybir.AluOpType.add,
                        )
                        if it < K_TAYLOR:
                            nc.scalar.copy(xnext[:, sI, :], out_ps[:, sI, :])
                            if sI == 3:
                                # first half-slab done: publish collective A
                                nc.sync.dma_start(
                                    out=slcA,
                                    in_=xnext[:, 0:4, :].rearrange(
                                        "p i f -> p (i f)"
                                    ),
                                )
                                nc.gpsimd.collective_compute(
                                    "AllGather",
                                    mybir.AluOpType.bypass,
                                    replica_groups=groups,
                                    ins=[slcA],
                                    outs=[xgA],
                                )
                            elif sI == NI - 1:
                                nc.sync.dma_start(
                                    out=slcB,
                                    in_=xnext[:, 4:8, :].rearrange(
                                        "p i f -> p (i f)"
                                    ),
                                )
                                nc.gpsimd.collective_compute(
                                    "AllGather",
                                    mybir.AluOpType.bypass,
                                    replica_groups=groups,
                                    ins=[slcB],
                                    outs=[xgB],
                                )

            nc.sync.dma_start(
                out=out_d.rearrange("(j p) f -> p j f", p=128),
                in_=result[:],
            )

    nc.compile()
    return nc


# ------------------------------------------------------------------ driver

_CACHE = {}


def _get_program(cfg: Cfg):
    if cfg not in _CACHE:
        _CACHE[cfg] = _build_program(cfg)
    return _CACHE[cfg]


def run(h, e, src, dst, cfg: Cfg = Cfg(), trace: bool = False):
    """Full pipeline: preprocess, build/compile (cached), execute, assemble."""
    h = np.asarray(h, dtype=np.float32)
    e = np.asarray(e, dtype=np.float32)
    src = np.asarray(src)
    dst = np.asarray(dst)
    nheads = e.shape[0]
    n = h.shape[0]
    dd = h.shape[1] // nheads
    assert (n, nheads, dd) == (cfg.n, H, d), (n, nheads, dd)

    tables = _make_tables(e, src, dst, cfg)
    x0 = np.ascontiguousarray(
        h.reshape(nheads, n, dd).transpose(1, 0, 2).reshape(n, nheads * dd)
    )
    # (128, NJ, D) fp16 in logical column order, partition-major
    x0p = np.ascontiguousarray(
        x0.astype(np.float16).reshape(NJ, 128, D)[_PHYS_J].transpose(1, 0, 2)
    ).reshape(128, NJ * D)
    rpc = cfg.rows_per_core
    in_maps = [
        {
            "x0p": x0p,
            "x0s": np.ascontiguousarray(x0[k * rpc : (k + 1) * rpc]),
            "goh": t["goh"],
            "sca": t["sca"],
            "w4": t["w4"],
        }
        for k, t in enumerate(tables)
    ]
    nc = _get_program(cfg)
    res = run_bass_kernel_spmd(
        nc,
        in_maps,
        list(range(cfg.n_cores)),
        trace=trace,
    )
    out = np.concatenate(
        [res.results[k]["out"] for k in range(cfg.n_cores)], axis=0
    )
    # back to reference layout: (n, H, d) node-major -> (H, n, d) -> (N, D)
    out = np.ascontiguousarray(out.reshape(n, nheads, dd).transpose(1, 0, 2)).reshape(
        n, nheads * dd
    )
    return out, res


def kernel(h, e, src, dst):
    out, _ = run(h, e, src, dst)
    return out


# revision 20
# speedup vs baseline: 7.3660x; 1.2673x over previous
"""Trainium2 Bass kernel for nn_LinearDiffusion (truncated Taylor expm(a) @ x).

Math: a = row-normalized symmetric scatter of per-head edge weights onto an
(H, N, N) zero tensor; result = sum_{i=0..6} a^i x / i! with x = h reshaped
per-head.

Strategy (8 NeuronCores, one chip), v2 — all-TensorE sparse SpMM:
  * The previous version gathered x[src] rows via SWDGE dma_gather: ~8 ns
    of Q7 descriptor generation per row x 32k rows x 6 iters dominated
    (GpSimd 91% busy).  v2 performs both the gather AND the scatter as
    one-hot matmuls on the tensor engine, which this problem leaves idle.
  * Layout: x node-major in SBUF as (128, 64, 64) fp16 — partition p,
    src block J, feature f (all 4 heads concatenated, d=16 each).
  * Edges are grouped by (I = dst block within core, J = src block) into
    8*64 = 512 "cells" per core, each holding <= 128 edges (max 96 for
    this graph).  The cell grid is the same for every core, so one SPMD
    program serves all 8 cores; only the one-hot tables differ per core.
  * Per cell (I, J), slot q holds one edge (r, c, w):
      1. gather:  ps[q, f]   = sum_p goh[p, q] * x[J*128+p, f]   (TensorE)
      2. weight:  xgw[q, f]  = ps[q, f] * w4[q, head(f)]         (VectorE,
         one op per group of 8 cells, PSUM -> SBUF fp16)
      3. scatter: out[m, f] += sum_q sca[q, m] * xgw[q, f]       (TensorE,
         PSUM-accumulated over the 64 J cells of block I)
  * Software-pipelined with a 2-group lookahead so the PE never waits on
    the vector engine; everything stays HAM-warm.
  * Taylor accumulation in fp32; x travels between iterations as fp16
    through an AllGather (1 MB) + strided SBUF reload.
"""

import math
from dataclasses import dataclass

import numpy as np

import concourse.bass as bass  # noqa: F401  (kept for callers)
import concourse.tile as tile
from concourse import bacc, mybir
from concourse.bass_utils import run_bass_kernel_spmd

# ----------------------------------------------------------------- config

N, H, E, D = 8192, 4, 131072, 64
d = D // H
NCORES = 8
BLK = 128
NJ = N // BLK          # 64 src blocks
NI = N // NCORES // BLK  # 8 dst blocks per core
NCELL = NI * NJ        # 512 cells per core
GRP = 8                # cells per PSUM bank / vector op
NGRP = NCELL // GRP    # 64 groups per iteration
LOOKAHEAD = 3          # groups the gather runs ahead of the scatter
K_TAYLOR = 6


@dataclass(frozen=True)
class Cfg:
    n: int = N
    n_cores: int = NCORES
    hi_lo_split: bool = True  # accepted for test.py compat; unused in v2

    @property
    def rows_per_core(self):
        return self.n // self.n_cores


# ----------------------------------------------------------- preprocessing


def _entries(e, src, dst, n):
    """Unique symmetric entries with 'last write wins' duplicate semantics,
    matching jax's .at[].set() on CPU. Returns (rows, cols, w[H, nnz])."""
    src = src.astype(np.int64)
    dst = dst.astype(np.int64)
    n_edges = len(src)
    keys = np.concatenate([src * n + dst, dst * n + src])
    eid = np.concatenate([np.arange(n_edges), np.arange(n_edges)])
    order = np.arange(2 * n_edges)
    perm = np.lexsort((-order, keys))
    k_sorted = keys[perm]
    first = np.ones(len(k_sorted), dtype=bool)
    first[1:] = k_sorted[1:] != k_sorted[:-1]
    win = perm[first]
    ukeys = k_sorted[first]
    rows = (ukeys // n).astype(np.int64)
    cols = (ukeys % n).astype(np.int64)
    weids = eid[win]
    vals = e[:, weids].astype(np.float64)  # (H, nnz)
    nheads = e.shape[0]
    rowsum = np.zeros((nheads, n), dtype=np.float64)
    for hh in range(nheads):
        rowsum[hh] = np.bincount(rows, weights=vals[hh], minlength=n)
    w = (vals / rowsum[:, rows]).astype(np.float32)
    return rows, cols, w


# Logical column order: the 4 blocks each core publishes in collective A
# (its dst blocks 0..3) come first, its collective-B blocks second.  This
# lets the next iteration consume the early-collective data first.
_PHYS_J = np.array(
    [8 * k + i for k in range(NCORES) for i in range(4)]
    + [8 * k + 4 + i for k in range(NCORES) for i in range(4)]
)


def _logical_j(J):
    k, i = J // NI, J % NI
    return np.where(i < 4, k * 4 + i, 32 + k * 4 + (i - 4))


def _make_tables(e, src, dst, cfg: Cfg):
    """Per-core one-hot gather/scatter tables for the (I, J) cell grid."""
    import ml_dtypes

    n = cfg.n
    rows, cols, w = _entries(e, src, dst, n)
    rpc = cfg.rows_per_core

    tables = []
    for k in range(cfg.n_cores):
        m = (rows >= k * rpc) & (rows < (k + 1) * rpc)
        r = rows[m] - k * rpc
        c = cols[m]
        wv = w[:, m]  # (H, nk)
        I = r // BLK
        J = _logical_j(c // BLK)
        cell = I * NJ + J
        order = np.argsort(cell, kind="stable")
        r, c, wv, cell = r[order], c[order], wv[:, order], cell[order]
        starts = np.searchsorted(cell, np.arange(NCELL + 1))
        counts = np.diff(starts)
        assert counts.max() <= 128, f"cell overflow: {counts.max()}"
        slot = np.arange(len(r)) - starts[cell]

        goh = np.zeros((128, NCELL * 128), dtype=ml_dtypes.float8_e4m3fn)
        goh[c % BLK, cell * 128 + slot] = 1.0
        sca = np.zeros((128, NCELL * 128), dtype=ml_dtypes.float8_e4m3fn)
        sca[slot, cell * 128 + (r % BLK)] = 1.0
        w4 = np.zeros((128, NCELL, H), dtype=np.float32)
        w4[slot, cell, :] = wv.T
        tables.append(
            {
                "goh": goh,
                "sca": sca,
                "w4": np.ascontiguousarray(w4.reshape(128, NCELL * H)),
            }
        )
    return tables


# ------------------------------------------------------------ bass program

_FP32 = mybir.dt.float32
_FP16 = mybir.dt.float16
_FP8 = mybir.dt.float8e4


def _build_program(cfg: Cfg):
    rpc = cfg.rows_per_core
    nc = bacc.Bacc(
        "TRN2",
        target_bir_lowering=False,
        debug=False,
        num_devices=cfg.n_cores,
    )

    x0p_d = nc.dram_tensor("x0p", [128, NJ * D], _FP16, kind="ExternalInput").ap()
    x0s_d = nc.dram_tensor("x0s", [rpc, D], _FP32, kind="ExternalInput").ap()
    goh_d = nc.dram_tensor(
        "goh", [128, NCELL * 128], _FP8, kind="ExternalInput"
    ).ap()
    sca_d = nc.dram_tensor(
        "sca", [128, NCELL * 128], _FP8, kind="ExternalInput"
    ).ap()
    w4_d = nc.dram_tensor("w4", [128, NCELL * H], _FP32, kind="ExternalInput").ap()
    out_d = nc.dram_tensor("out", [rpc, D], _FP32, kind="ExternalOutput").ap()

    # half-slab collective buffers, rank-major so both sides stream linearly
    HD = 4 * D  # half-slab free size per partition (4 blocks x 64 feats)
    slcA = nc.dram_tensor("slcA", [128, HD], _FP16).ap()
    slcB = nc.dram_tensor("slcB", [128, HD], _FP16).ap()
    xgA = nc.dram_tensor(
        "xgA", [NCORES, 128, HD], _FP16, addr_space="Shared"
    ).ap()
    xgB = nc.dram_tensor(
        "xgB", [NCORES, 128, HD], _FP16, addr_space="Shared"
    ).ap()
    # tiny warmup collective to absorb ncfw first-call cost (data unused)
    slcW = nc.dram_tensor("slcW", [1, 128], _FP16).ap()
    xgW = nc.dram_tensor("xgW", [NCORES, 128], _FP16, addr_space="Shared").ap()

    groups = [list(range(cfg.n_cores))]
    CPI = NJ * 128  # table columns per dst block I

    with tile.TileContext(nc) as tc:
        with (
            tc.tile_pool(name="tables", bufs=1) as tp,
            tc.tile_pool(name="xall", bufs=2) as xap,
            tc.tile_pool(name="xgw", bufs=6) as xgp,
            tc.tile_pool(name="acc", bufs=1) as accp,
            tc.tile_pool(name="xnext", bufs=2) as xnp,
            tc.tile_pool(name="psg", bufs=6, space="PSUM") as pgp,
            tc.tile_pool(name="pso", bufs=1, space="PSUM") as pop,
        ):
            goh_t = [
                tp.tile([128, CPI], _FP8, name=f"goh{i}") for i in range(NI)
            ]
            sca_t = [
                tp.tile([128, CPI], _FP8, name=f"sca{i}") for i in range(NI)
            ]
            w4_t = [
                tp.tile([128, NJ, H], _FP32, name=f"w4{i}") for i in range(NI)
            ]
            result = accp.tile([128, NI, D], _FP32)

            # warm up the collective firmware while tables stream in
            nc.gpsimd.collective_compute(
                "AllGather",
                mybir.AluOpType.bypass,
                replica_groups=groups,
                ins=[slcW],
                outs=[xgW],
            )

            # load order = consumption order so compute starts early
            nc.sync.dma_start(out=goh_t[0][:], in_=goh_d[:, 0:CPI])
            xallA = xap.tile([128, NJ // 2, D], _FP16, tag="xa")
            xallB = xap.tile([128, NJ // 2, D], _FP16, tag="xb")
            nc.sync.dma_start(
                out=xallA[:],
                in_=x0p_d[:, 0 : NJ * D // 2].rearrange("p (j f) -> p j f", f=D),
            )
            nc.sync.dma_start(
                out=xallB[:],
                in_=x0p_d[:, NJ * D // 2 :].rearrange("p (j f) -> p j f", f=D),
            )
            nc.sync.dma_start(
                out=w4_t[0][:].rearrange("p c h -> p (c h)"),
                in_=w4_d[:, 0 : NJ * H],
            )
            nc.sync.dma_start(out=sca_t[0][:], in_=sca_d[:, 0:CPI])
            nc.sync.dma_start(
                out=result[:],
                in_=x0s_d.rearrange("(j p) f -> p j f", p=128),
            )
            for i in range(1, NI):
                nc.sync.dma_start(
                    out=goh_t[i][:], in_=goh_d[:, i * CPI : (i + 1) * CPI]
                )
                nc.sync.dma_start(
                    out=w4_t[i][:].rearrange("p c h -> p (c h)"),
                    in_=w4_d[:, i * NJ * H : (i + 1) * NJ * H],
                )
                nc.sync.dma_start(
                    out=sca_t[i][:], in_=sca_d[:, i * CPI : (i + 1) * CPI]
                )

            out_psA = pop.tile([128, NI, D], _FP32, name="opsA")
            out_psB = pop.tile([128, NI, D], _FP32, name="opsB")

            for it in range(1, K_TAYLOR + 1):
                coef = 1.0 / math.factorial(it)
                if it > 1:
                    xallA = xap.tile([128, NJ // 2, D], _FP16, tag="xa")
                    xallB = xap.tile([128, NJ // 2, D], _FP16, tag="xb")
                    nc.sync.dma_start(
                        out=xallA[:].rearrange("p (k i) f -> p k i f", k=NCORES),
                        in_=xgA.rearrange("k p (i f) -> p k i f", f=D),
                    )
                    nc.sync.dma_start(
                        out=xallB[:].rearrange("p (k i) f -> p k i f", k=NCORES),
                        in_=xgB.rearrange("k p (i f) -> p k i f", f=D),
                    )
                xnext = xnp.tile([128, NI, D], _FP16, tag="xnext")

                # software-pipelined gather -> weight -> scatter over groups.
                # Phase 1 = all groups touching collective-A columns (PSUM
                # bank A), phase 2 = collective-B columns (bank B) — so the
                # next iteration's wait on collective B is tiny.  Each bank's
                # per-I accumulation group stays contiguous.
                glist = [
                    (I, g)
                    for half in range(2)
                    for I in range(NI)
                    for g in range(half * 4, half * 4 + 4)
                ]
                pend = [None] * NGRP  # xgw tiles awaiting scatter
                for t in range(NGRP + LOOKAHEAD):
                    if t < NGRP:
                        gI, gg = glist[t]
                        ps_g = pgp.tile([128, GRP, D], _FP32, tag="psg")
                        for jj in range(GRP):
                            J = gg * GRP + jj
                            xsrc = (
                                xallA[:, J, :]
                                if J < NJ // 2
                                else xallB[:, J - NJ // 2, :]
                            )
                            cc = J * 128
                            nc.tensor.matmul(
                                ps_g[:, jj, :],
                                lhsT=goh_t[gI][:, cc : cc + 128],
                                rhs=xsrc,
                                start=True,
                                stop=True,
                            )
                        xgw_g = xgp.tile([128, GRP, D], _FP16, tag="xgw")
                        c0 = gg * GRP
                        wv = (
                            w4_t[gI][:, c0 : c0 + GRP, :]
                            .unsqueeze(3)
                            .to_broadcast([128, GRP, H, d])
                        )
                        nc.vector.tensor_mul(
                            xgw_g[:].rearrange("p j (h f) -> p j h f", h=H),
                            ps_g[:].rearrange("p j (h f) -> p j h f", h=H),
                            wv,
                        )
                        pend[t] = xgw_g
                    s = t - LOOKAHEAD
                    if s < 0:
                        continue
                    sI, sg = glist[s]
                    xgw_s = pend[s]
                    pend[s] = None
                    bank = out_psA if sg < 4 else out_psB
                    for jj in range(GRP):
                        J = sg * GRP + jj
                        cc = J * 128
                        nc.tensor.matmul(
                            bank[:, sI, :],
                            lhsT=sca_t[sI][:, cc : cc + 128],
                            rhs=xgw_s[:, jj, :],
                            start=(J % (NJ // 2) == 0),
                            stop=(J % (NJ // 2) == NJ // 2 - 1),
                        )
                    if sg == NGRP // NI - 1:
                        # block sI complete: combine banks, Taylor accumulate
                        nc.scalar.copy(xnext[:, sI, :], out_psA[:, sI, :])
                        nc.vector.tensor_add(
                            xnext[:, sI, :],
                            xnext[:, sI, :],
                            out_psB[:, sI, :],
                        )
                        nc.vector.scalar_tensor_tensor(
                            result[:, sI, :],
                            xnext[:, sI, :],
                            coef,
                            result[:, sI, :],
                            op0=mybir.AluOpType.mult,
                            op1=mybir.AluOpType.add,
                        )
                        if it < K_TAYLOR:
                            if sI == 3:
                                # first half-slab done: publish collective A
                                nc.sync.dma_start(
                                    out=slcA,
                                    in_=xnext[:, 0:4, :].rearrange(
                                        "p i f -> p (i f)"
                                    ),
                                )
                                nc.gpsimd.collective_compute(
                                    "AllGather",
                                    mybir.AluOpType.bypass,
                                    replica_groups=groups,
                                    ins=[slcA],
                                    outs=[xgA],
                                )
                            elif sI == NI - 1:
                                nc.sync.dma_start(
                                    out=slcB,
                                    in_=xnext[:, 4:8, :].rearrange(
                                        "p i f -> p (i f)"
                                    ),
                                )
                                nc.gpsimd.collective_compute(
                                    "AllGather",
                                    mybir.AluOpType.bypass,
                                    replica_groups=groups,
                                    ins=[slcB],
                                    outs=[xgB],
                                )

            nc.sync.dma_start(
                out=out_d.rearrange("(j p) f -> p j f", p=128),
                in_=result[:],
            )

    nc.compile()
    return nc


# ------------------------------------------------------------------ driver

_CACHE = {}


def _get_program(cfg: Cfg):
    if cfg not in _CACHE:
        _CACHE[cfg] = _build_program(cfg)
    return _CACHE[cfg]


def run(h, e, src, dst, cfg: Cfg = Cfg(), trace: bool = False):
    """Full pipeline: preprocess, build/compile (cached), execute, assemble."""
    h = np.asarray(h, dtype=np.float32)
    e = np.asarray(e, dtype=np.float32)
    src = np.asarray(src)
    dst = np.asarray(dst)
    nheads = e.shape[0]
    n = h.shape[0]
    dd = h.shape[1] // nheads
    assert (n, nheads, dd) == (cfg.n, H, d), (n, nheads, dd)

    tables = _make_tables(e, src, dst, cfg)
    x0 = np.ascontiguousarray(
        h.reshape(nheads, n, dd).transpose(1, 0, 2).reshape(n, nheads * dd)
    )
    # (128, NJ, D) fp16 in logical column order, partition-major
    x0p = np.ascontiguousarray(
        x0.astype(np.float16).reshape(NJ, 128, D)[_PHYS_J].transpose(1, 0, 2)
    ).reshape(128, NJ * D)
    rpc = cfg.rows_per_core
    in_maps = [
        {
            "x0p": x0p,
            "x0s": np.ascontiguousarray(x0[k * rpc : (k + 1) * rpc]),
            "goh": t["goh"],
            "sca": t["sca"],
            "w4": t["w4"],
        }
        for k, t in enumerate(tables)
    ]
    nc = _get_program(cfg)
    res = run_bass_kernel_spmd(
        nc,
        in_maps,
        list(range(cfg.n_cores)),
        trace=trace,
    )
    out = np.concatenate(
        [res.results[k]["out"] for k in range(cfg.n_cores)], axis=0
    )
    # back to reference layout: (n, H, d) node-major -> (H, n, d) -> (N, D)
    out = np.ascontiguousarray(out.reshape(n, nheads, dd).transpose(1, 0, 2)).reshape(
        n, nheads * dd
    )
    return out, res


def kernel(h, e, src, dst):
    out, _ = run(h, e, src, dst)
    return out
